# revision 1
# baseline (speedup 1.0000x reference)
"""DGCNN (6x dynamic EdgeConv + MLP head) Trainium2 Bass kernel.

Data-parallel over the 64 graphs: 8 graphs per NeuronCore x 8 cores.

Per graph (N=1024 nodes, K=10 neighbours), each EdgeConv(F -> 64):
  score_ij = h_i . h_j - 0.5|h_j|^2   (row order == -dist order; one PE
             matmul per 128-row block with a folded [h;-0.5|h|^2]/[h;ones] row)
  top-10 per row via DVE max8 / max_index / match_replace (exact, ties to
             lower index like jax.lax.top_k)
  msg_ij = lrelu(u_i + v_j),  u = h(Wa-Wb)+b, v = h Wb (node-major via PE);
             v rows staged to DRAM and gathered by hardware DGE dma_gather
             (10x 1024-descriptor chunks; 16-partition-wrapped index list
             built by 8 small element-granular DMAs)
  h'_i = sum_k msg (DVE in-place tree sum), PE-transposed back to [64, N].
"""
import numpy as np

B, N, K = 64, 1024, 10
NCORES = 8
G = B // NCORES          # graphs per core
F1, F, C = 5, 64, 64     # conv1 in-feat, conv2+ in-feat, out-feat
F1P = 32                 # conv1 feature rows padded (engine partition alignment)
NBLK = N // 128          # 8 row blocks per graph
NEG = -1.0e30

_COMPILED = {}


def _build_program(debug_taps=False, loop_reps=0):
    from contextlib import ExitStack
    import concourse.bass as bass
    import concourse.tile as tile
    import concourse.mybir as mybir
    from concourse import bacc
    from concourse.masks import make_identity

    fp32 = mybir.dt.float32
    u32 = mybir.dt.uint32
    i16 = mybir.dt.int16
    AX = mybir.AxisListType
    ALU = mybir.AluOpType
    ACTF = mybir.ActivationFunctionType

    nc = bacc.Bacc("TRN2", target_bir_lowering=False, debug=False)

    # ---------------- DRAM I/O ----------------
    xxT_d = nc.dram_tensor("xxT", [F1, G * N], fp32, kind="ExternalInput")
    WD1_d = nc.dram_tensor("WD1", [F1P + 1, C], fp32, kind="ExternalInput")
    WB1_d = nc.dram_tensor("WB1", [F1P, C], fp32, kind="ExternalInput")
    WD2_d = nc.dram_tensor("WD2", [F + 1, C], fp32, kind="ExternalInput")
    WB2_d = nc.dram_tensor("WB2", [F, C], fp32, kind="ExternalInput")
    WL1A_d = nc.dram_tensor("WL1A", [384, 512], fp32, kind="ExternalInput")
    BL1A_d = nc.dram_tensor("BL1A", [512], fp32, kind="ExternalInput")
    WL1B_d = nc.dram_tensor("WL1B", [512, 512], fp32, kind="ExternalInput")
    BL1B_d = nc.dram_tensor("BL1B", [512], fp32, kind="ExternalInput")
    WM1_d = nc.dram_tensor("WM1", [512, 256], fp32, kind="ExternalInput")
    BM1_d = nc.dram_tensor("BM1", [256], fp32, kind="ExternalInput")
    WM2_d = nc.dram_tensor("WM2", [256, 3], fp32, kind="ExternalInput")
    BM2_d = nc.dram_tensor("BM2", [3], fp32, kind="ExternalInput")
    OUT_d = nc.dram_tensor("out", [3, G], fp32, kind="ExternalOutput")
    taps = {}
    if debug_taps:
        taps["idx1"] = nc.dram_tensor("tap_idx1", [128, 128], u32, kind="ExternalOutput")
        taps["idxg1"] = nc.dram_tensor("tap_idxg1", [128, 640], i16, kind="ExternalOutput")
        taps["u1"] = nc.dram_tensor("tap_u1", [128, 512], fp32, kind="ExternalOutput")
        taps["v1"] = nc.dram_tensor("tap_v1", [128, 512], fp32, kind="ExternalOutput")
        taps["vg1"] = nc.dram_tensor("tap_vg1", [128, 5120], fp32, kind="ExternalOutput")
        taps["h1"] = nc.dram_tensor("tap_h1", [64, 1024], fp32, kind="ExternalOutput")

    with tile.TileContext(nc) as tc, ExitStack() as ctx:
        consts = ctx.enter_context(tc.tile_pool(name="consts", bufs=1))
        p_h = ctx.enter_context(tc.tile_pool(name="hT", bufs=4))
        p_hl = ctx.enter_context(tc.tile_pool(name="hl", bufs=2))
        p_hsq = ctx.enter_context(tc.tile_pool(name="hsq", bufs=2))
        p_sc2 = ctx.enter_context(tc.tile_pool(name="sc2", bufs=2))
        p_v8 = ctx.enter_context(tc.tile_pool(name="v8", bufs=3))
        p_idx = ctx.enter_context(tc.tile_pool(name="idx", bufs=2))
        p_uv = ctx.enter_context(tc.tile_pool(name="uvsb", bufs=3))
        p_vg = ctx.enter_context(tc.tile_pool(name="vg", bufs=2))
        p_hc = ctx.enter_context(tc.tile_pool(name="hc", bufs=2))
        p_y0 = ctx.enter_context(tc.tile_pool(name="y0", bufs=1))
        p_small = ctx.enter_context(tc.tile_pool(name="small", bufs=4))

        ps_sc = ctx.enter_context(tc.tile_pool(name="ps_sc", bufs=2, space="PSUM"))
        ps_small = ctx.enter_context(tc.tile_pool(name="ps_small", bufs=4, space="PSUM"))

        dram = ctx.enter_context(tc.tile_pool(name="vscr", bufs=2, space="DRAM"))

        # ---------------- constants / weights ----------------
        ident = consts.tile([128, 128], fp32)
        make_identity(nc, ident[:, :])

        wd1 = consts.tile([F1P + 1, C], fp32)
        nc.sync.dma_start(wd1[:, :], WD1_d[:, :])
        wb1 = consts.tile([F1P, C], fp32)
        nc.sync.dma_start(wb1[:, :], WB1_d[:, :])
        wd2 = consts.tile([F + 1, C], fp32)
        nc.sync.dma_start(wd2[:, :], WD2_d[:, :])
        wb2 = consts.tile([F, C], fp32)
        nc.sync.dma_start(wb2[:, :], WB2_d[:, :])

        wl1a = [consts.tile([128, 512], fp32, tag=f"wl1a{k}", name=f"wl1a{k}") for k in range(3)]
        for k in range(3):
            nc.sync.dma_start(wl1a[k][:, :], WL1A_d[k * 128 : (k + 1) * 128, :])
        wl1b = [consts.tile([128, 512], fp32, tag=f"wl1b{k}", name=f"wl1b{k}") for k in range(4)]
        for k in range(4):
            nc.sync.dma_start(wl1b[k][:, :], WL1B_d[k * 128 : (k + 1) * 128, :])
        wm1 = [consts.tile([128, 256], fp32, tag=f"wm1{k}", name=f"wm1{k}") for k in range(4)]
        for k in range(4):
            nc.sync.dma_start(wm1[k][:, :], WM1_d[k * 128 : (k + 1) * 128, :])
        wm2 = [consts.tile([128, 3], fp32, tag=f"wm2{k}", name=f"wm2{k}") for k in range(2)]
        for k in range(2):
            nc.sync.dma_start(wm2[k][:, :], WM2_d[k * 128 : (k + 1) * 128, :])

        bl1a = consts.tile([128, 4], fp32)
        bl1b = consts.tile([128, 4], fp32)
        bm1 = consts.tile([128, 2], fp32)
        bm2 = consts.tile([3, 1], fp32)
        for o in range(4):
            nc.sync.dma_start(bl1a[:, o : o + 1], BL1A_d[o * 128 : (o + 1) * 128].unsqueeze(1))
            nc.sync.dma_start(bl1b[:, o : o + 1], BL1B_d[o * 128 : (o + 1) * 128].unsqueeze(1))
        for o in range(2):
            nc.sync.dma_start(bm1[:, o : o + 1], BM1_d[o * 128 : (o + 1) * 128].unsqueeze(1))
        nc.sync.dma_start(bm2[:, :], BM2_d[:].unsqueeze(1))

        neghalf = consts.tile([F, 1], fp32)
        nc.gpsimd.memset(neghalf[:, :], -0.5)

        pooled = [consts.tile([128, G], fp32, tag=f"pool{k}", name=f"pool{k}") for k in range(4)]

        # ---------------- one EdgeConv ----------------
        def edge_conv(g, t, hin, Fin, wd, wb):
            """hin [Fin+1, N]: rows 0:Fin = h^T; row Fin gets -0.5|h|^2 here.
            Returns hout [F+1, N] (rows 0:64 = h'^T)."""
            hsq = p_hsq.tile([F, N], fp32, tag="hsq" if Fin == F else "hsq1",
                             name="hsq")
            nc.scalar.square(hsq[0:Fin, :], hin[0:Fin, :])
            for hf in range(2):
                sl = slice(hf * 512, (hf + 1) * 512)
                nh_ps = ps_small.tile([1, 512], fp32, tag="ps1b", name="nh_ps")
                nc.tensor.matmul(nh_ps[:, :], neghalf[0:Fin, :], hsq[0:Fin, sl],
                                 start=True, stop=True)
                nc.scalar.copy(hin[Fin : Fin + 1, sl], nh_ps[:, :])

            hl = p_hl.tile([Fin + 1, N], fp32, tag="hl" if Fin == F else "hl1",
                           name="hl")
            nc.scalar.copy(hl[0:Fin, :], hin[0:Fin, :])
            nc.gpsimd.memset(hl[Fin : Fin + 1, :], 1.0)

            # u/v node-major (bias folded into u via HL ones row)
            u_ps = ps_small.tile([128, 512], fp32, tag="ps1b", name="u_ps")
            v_ps = ps_small.tile([128, 512], fp32, tag="ps1b", name="v_ps")
            for blk in range(NBLK):
                lhs = hl[:, blk * 128 : (blk + 1) * 128]
                cs = slice(blk * C, (blk + 1) * C)
                nc.tensor.matmul(u_ps[:, cs], lhs, wd[:, :], start=True, stop=True)
                nc.tensor.matmul(v_ps[:, cs], lhs[0:Fin, :], wb[:, :], start=True, stop=True)
            u_sb = p_uv.tile([128, 512], fp32, tag="usb", name="u_sb")
            nc.scalar.copy(u_sb[:, :], u_ps[:, :])
            v_sb = p_uv.tile([128, 512], fp32, tag="vsb", name="v_sb")
            nc.scalar.copy(v_sb[:, :], v_ps[:, :])
            v_scr = dram.tile([N, C], fp32, tag="vscr", name="v_scr")
            nc.sync.dma_start(
                v_scr[:, :].rearrange("(blk p) c -> p blk c", p=128), v_sb[:, :]
            )

            # scores + top-10 per 128-row block
            idx = p_idx.tile([128, NBLK * 16], u32, tag="idx", name="idx")
            for blk in range(NBLK):
                sc_ps = ps_sc.tile([128, N], fp32, tag="sc", name="sc_ps")
                for hf in range(2):
                    sl = slice(hf * 512, (hf + 1) * 512)
                    nc.tensor.matmul(
                        sc_ps[:, sl], hl[:, blk * 128 : (blk + 1) * 128],
                        hin[:, sl], start=True, stop=True)
                v8 = p_v8.tile([128, 8], fp32, tag="v8", name="v8")
                nc.vector.max(v8[:, :], sc_ps[:, :])
                nc.vector.max_index(idx[:, blk * 16 : blk * 16 + 8], v8[:, :], sc_ps[:, :])
                sc2 = p_sc2.tile([128, N], fp32, tag="sc2", name="sc2")
                nc.vector.match_replace(sc2[:, :], v8[:, :], sc_ps[:, :], NEG)
                v8b = p_v8.tile([128, 8], fp32, tag="v8", name="v8b")
                nc.vector.max(v8b[:, :], sc2[:, :])
                nc.vector.max_index(idx[:, blk * 16 + 8 : blk * 16 + 16], v8b[:, :], sc2[:, :])

            idx16 = p_idx.tile([128, NBLK * K], i16, tag="idx16", name="idx16")
            nc.vector.tensor_copy(
                idx16[:, :].rearrange("p (blk k) -> p blk k", k=K),
                idx[:, :].rearrange("p (blk s) -> p blk s", s=16)[:, :, 0:K],
            )

            # wrap for dma_gather: row m = q*128 + i_lo lands on partition
            # i_lo; its index must sit at (p_w = m%16, slot = q*8 + i_lo//16).
            idxg = p_idx.tile([128, N * K // 16], i16, tag="idxg", name="idxg")
            for h8 in range(8):
                nc.scalar.dma_start(
                    idxg[0:16, :].rearrange("p (q h) -> p q h", h=8)
                    [:, :, h8 : h8 + 1],
                    idx16[16 * h8 : 16 * (h8 + 1), :].unsqueeze(2),
                )
            nc.sync.dma_start(idxg[16:32, :], idxg[0:16, :])
            nc.sync.dma_start(idxg[32:64, :], idxg[0:32, :])
            nc.sync.dma_start(idxg[64:128, :], idxg[0:64, :])

            # gather v rows: vg[i_lo, q=(blk*10+k), c] = v[idx(i, k), c]
            vg = p_vg.tile([128, NBLK * K * C], fp32, tag="vg", name="vg")
            for ch in range(10):
                nc.gpsimd.dma_gather(
                    vg[:, ch * 512 : (ch + 1) * 512].rearrange("p (q c) -> p q c", c=C),
                    v_scr[:, :], idxg[:, ch * 64 : (ch + 1) * 64],
                    num_idxs=1024, num_idxs_reg=1024, elem_size=C,
                )

            # z = vg + u_i ; lrelu ; in-place tree sum over K
            u_bc = (
                u_sb[:, :]
                .rearrange("p (blk c) -> p blk c", blk=NBLK)
                .unsqueeze(2)
                .to_broadcast([128, NBLK, K, C])
            )
            vg4 = vg[:, :].rearrange("p (blk k c) -> p blk k c", blk=NBLK, k=K)
            nc.vector.tensor_add(vg4, vg4, u_bc)
            nc.scalar.activation(vg[:, :], vg[:, :], ACTF.Lrelu, alpha=0.01)
            # tree sum over k: 10 -> 5 -> (2+2+1) -> 1  (contiguous chunks)
            nc.vector.tensor_add(vg4[:, :, 0:5, :], vg4[:, :, 0:5, :], vg4[:, :, 5:10, :])
            nc.vector.tensor_add(vg4[:, :, 0:2, :], vg4[:, :, 0:2, :], vg4[:, :, 2:4, :])
            nc.vector.tensor_add(vg4[:, :, 0:1, :], vg4[:, :, 0:1, :], vg4[:, :, 1:2, :])
            nc.vector.tensor_add(vg4[:, :, 0:1, :], vg4[:, :, 0:1, :], vg4[:, :, 4:5, :])

            # transpose h' back to channel-major [64, N]
            hout = p_h.tile([F + 1, N], fp32, tag="hT", name="hout")
            for half in range(2):
                htp = ps_small.tile([C, 512], fp32, tag="ps1b", name="htp")
                for b4 in range(4):
                    blk = half * 4 + b4
                    nc.tensor.transpose(
                        htp[:, b4 * 128 : (b4 + 1) * 128],
                        vg4[:, blk, 0, :],
                        ident[:, :],
                    )
                nc.scalar.copy(hout[0:C, half * 512 : (half + 1) * 512], htp[:, :])

            if debug_taps and g == 0 and t == 0:
                nc.sync.dma_start(taps["idx1"][:, :], idx[:, :])
                nc.sync.dma_start(taps["idxg1"][:, :], idxg[:, :])
                nc.sync.dma_start(taps["u1"][:, :], u_sb[:, :])
                nc.sync.dma_start(taps["v1"][:, :], v_sb[:, :])
                nc.sync.dma_start(taps["vg1"][:, :], vg[:, :])
                nc.sync.dma_start(taps["h1"][:, :], hout[0:C, :])
            return hout

        # ---------------- MLP per graph ----------------
        def mlp(g, hcg):
            y0 = [p_y0.tile([128, N], fp32, tag=f"y0{o}", name=f"y0{o}") for o in range(4)]
            for o in range(4):
                o_ps = ps_sc.tile([128, N], fp32, tag="sc", name="o_ps")
                for k in range(3):
                    for hf in range(2):
                        sl = slice(hf * 512, (hf + 1) * 512)
                        nc.tensor.matmul(
                            o_ps[:, sl], wl1a[k][:, o * 128 : (o + 1) * 128],
                            hcg[k][:, sl], start=(k == 0), stop=(k == 2))
                nc.scalar.activation(
                    y0[o][:, :], o_ps[:, :], ACTF.Lrelu,
                    bias=bl1a[:, o : o + 1], alpha=0.01)
            for o in range(4):
                y_ps = ps_sc.tile([128, N], fp32, tag="sc", name="y_ps")
                for k in range(4):
                    for hf in range(2):
                        sl = slice(hf * 512, (hf + 1) * 512)
                        nc.tensor.matmul(
                            y_ps[:, sl], wl1b[k][:, o * 128 : (o + 1) * 128],
                            y0[k][:, sl], start=(k == 0), stop=(k == 3))
                pm = p_small.tile([128, 1], fp32, tag="pm", name="pm")
                nc.vector.tensor_reduce(pm[:, :], y_ps[:, :], axis=AX.X, op=ALU.max)
                nc.scalar.activation(
                    pooled[o][:, g : g + 1], pm[:, :], ACTF.Identity,
                    bias=bl1b[:, o : o + 1])

        # ---------------- per-graph pipeline ----------------
        def full_body():
            for g in range(G):
                h0 = p_h.tile([F1P + 1, N], fp32, tag="h0", name="h0", bufs=2)
                nc.gpsimd.memset(h0[0:F1P, :], 0.0)
                nc.sync.dma_start(h0[0:F1, :], xxT_d[:, g * N : (g + 1) * N])

                hc = [p_hc.tile([128, N], fp32, tag=f"hcp{s}", name=f"hcp{s}")
                      for s in range(3)]

                hin = h0
                for t in range(6):
                    hout = edge_conv(
                        g, t, hin,
                        F1P if t == 0 else F,
                        wd1 if t == 0 else wd2,
                        wb1 if t == 0 else wb2,
                    )
                    nc.sync.dma_start(
                        hc[t // 2][(t % 2) * 64 : (t % 2) * 64 + 64, :],
                        hout[0:C, :],
                    )
                    hin = hout

                mlp(g, hc)

            # ---------------- head over all G graphs ----------------
            zh = [p_small.tile([128, G], fp32, tag=f"zh{o}", name=f"zh{o}")
                  for o in range(2)]
            for o in range(2):
                z_ps = ps_small.tile([128, G], fp32, tag="ps1b", name="z_ps")
                for k in range(4):
                    nc.tensor.matmul(
                        z_ps[:, :], wm1[k][:, o * 128 : (o + 1) * 128],
                        pooled[k][:, :], start=(k == 0), stop=(k == 3))
                nc.scalar.activation(
                    zh[o][:, :], z_ps[:, :], ACTF.Lrelu, bias=bm1[:, o : o + 1],
                    alpha=0.01)
            o_ps = ps_small.tile([3, G], fp32, tag="ps1b", name="o_ps2")
            for k in range(2):
                nc.tensor.matmul(o_ps[:, :], wm2[k][:, :], zh[k][:, :],
                                 start=(k == 0), stop=(k == 1))
            out_sb = p_small.tile([3, G], fp32, tag="osb", name="out_sb")
            nc.scalar.activation(out_sb[:, :], o_ps[:, :], ACTF.Identity,
                                 bias=bm2[:, :])
            nc.sync.dma_start(OUT_d[:, :], out_sb[:, :])

        if loop_reps:
            with tc.For_i(0, loop_reps, 1) as _i:
                full_body()
        else:
            full_body()

    nc.compile()
    return nc


def _pad_rows(w, brow):
    """Pad [5, C] weight rows to F1P rows of zeros; append bias row if given."""
    rows = F1P + (1 if brow is not None else 0)
    out = np.zeros((rows, w.shape[1]), np.float32)
    out[: w.shape[0]] = w
    if brow is not None:
        out[F1P] = brow
    return out


def _prep_inputs(x, pos, tq, W1, b1, W2, b2, Wl1a, bl1a, Wl1b, bl1b,
                 Wm1, bm1, Wm2, bm2):
    xx = np.concatenate([tq, x, pos], axis=1).astype(np.float32)  # [B*N, 5]
    base = {
        "WD1": _pad_rows(W1[0:F1] - W1[F1:2 * F1], b1),
        "WB1": _pad_rows(W1[F1:2 * F1], None),
        "WD2": np.ascontiguousarray(
            np.concatenate([W2[0:F] - W2[F:2 * F], b2[None, :]], 0), np.float32),
        "WB2": np.ascontiguousarray(W2[F:2 * F], np.float32),
        "WL1A": np.ascontiguousarray(Wl1a, np.float32),
        "BL1A": np.ascontiguousarray(bl1a, np.float32),
        "WL1B": np.ascontiguousarray(Wl1b, np.float32),
        "BL1B": np.ascontiguousarray(bl1b, np.float32),
        "WM1": np.ascontiguousarray(Wm1, np.float32),
        "BM1": np.ascontiguousarray(bm1, np.float32),
        "WM2": np.ascontiguousarray(Wm2, np.float32),
        "BM2": np.ascontiguousarray(bm2, np.float32),
    }
    in_maps = []
    for c in range(NCORES):
        m = dict(base)
        m["xxT"] = np.ascontiguousarray(
            xx[c * G * N : (c + 1) * G * N].T, np.float32)
        in_maps.append(m)
    return in_maps


def kernel(x, pos, tq, batch, W1, b1, W2, b2, Wl1a, bl1a, Wl1b, bl1b,
           Wm1, bm1, Wm2, bm2):
    """Full inputs -> full [B, 3] output, running on 8 NeuronCores."""
    from concourse.bass_utils import run_bass_kernel_spmd

    if "nc" not in _COMPILED:
        _COMPILED["nc"] = _build_program()
    nc = _COMPILED["nc"]
    in_maps = _prep_inputs(np.asarray(x), np.asarray(pos), np.asarray(tq),
                           np.asarray(W1), np.asarray(b1), np.asarray(W2),
                           np.asarray(b2), np.asarray(Wl1a), np.asarray(bl1a),
                           np.asarray(Wl1b), np.asarray(bl1b), np.asarray(Wm1),
                           np.asarray(bm1), np.asarray(Wm2), np.asarray(bm2))
    res = run_bass_kernel_spmd(nc, in_maps, core_ids=list(range(NCORES)))
    out = np.concatenate([res.results[c]["out"].T for c in range(NCORES)], axis=0)
    return out.astype(np.float32)

